# revision 31
# baseline (speedup 1.0000x reference)
"""CostVolume kernel for Trainium2 (8 NeuronCores, batch-sharded).

out[b,h,w,(di,dj)] = mean_c( prv[b,h,w,c] * nxt_pad[b,h+di,w+dj,c] ),  r=4, d=9.

Device strategy (per core, 2 batches):
  - Host prep: prv scaled by 1/C -> bf16, patch-major [b, c, I, J, 128];
    nxt -> bf16, [b, c, 128, 136] (cols zero-padded by 4; rows padded on
    device via gpsimd memset of the SBUF border).
  - Per 16x8-pixel patch: matmul (M=128 pixels, N=384 = 24x16 nxt window)
    contracting c: chunk1 K=128 (start) + chunk2 K=64 (accumulate). No
    operand duplication - PE issue rate is N-columns * cycle regardless of K.
  - Band SBUF layout [128, NB=384, NJ=16] (J innermost): PSUM->SBUF copies
    write J-strided; the windowed store per 32-partition group q reads
    band[32q:32q+32, 64q:64q+192, :] = one contiguous 6144B run per
    partition (full DMA efficiency), 12.6 MB/core instead of 25.2 MB.
  - PSUM->SBUF bf16 copies rotate across DVE / ACT / Pool so the tensor
    engine never stalls on PSUM recycling (lets the PE ramp past the
    1.2 GHz mid p-state to 2.4 GHz).
  - Rings: gpsimd (SWDGE) nxt loads + memsets, sync (SP) prv loads +
    band stores, scalar/vector/gpsimd share the copies.
  - Host gathers out from band8[b,I,q,p32,nb,J] with
    nb = ((p32>>3)+di)*16 + (p32&7) + dj.
"""

import numpy as np
import ml_dtypes

B, H, W, C = 16, 128, 128, 192
R = 4
D = 2 * R + 1  # 9
N_CORES = 8
B_LOC = B // N_CORES  # 2
# PE moving-read is quantized to ceil(K/64) beats per column, so split
# C=192 as K=128 (2 beats) + K=64 (1 beat) and group same-K matmuls.
C0 = 128
C1 = C - C0  # 64
PH, PW = 16, 8  # patch size (h, w); PH*PW = 128 = M
WH, WW = PH + 2 * R, PW + 2 * R  # 24, 16 window
NB = WH * WW  # 384 band columns per patch
NI = H // PH  # 8 patch rows
NJ = W // PW  # 16 patch cols
HP = H + 2 * R  # 136 padded rows (SBUF)
WP = W + 2 * R  # 136 padded cols (HBM + SBUF)
QW = 256  # band window width per 64-partition half (contiguous in SBUF)
NSL = 4  # h-slices per nxt load

_CACHED = {}


def _build_nc():
    import concourse.mybir as mybir
    from concourse.bacc import Bacc
    from concourse.tile import TileContext

    fp32 = mybir.dt.float32
    bf16 = mybir.dt.bfloat16

    nc = Bacc(
        "TRN2",
        target_bir_lowering=False,
        debug=False,
        num_devices=N_CORES,
    )

    prv_d = nc.dram_tensor(
        "prv_t", [B_LOC, C, NI, NJ, PH * PW], bf16, kind="ExternalInput"
    )
    nxt_d = nc.dram_tensor("nxt_p", [B_LOC, C, H, WP], bf16, kind="ExternalInput")
    band_d = nc.dram_tensor(
        "band", [B_LOC, NI, PH * PW, NJ, QW], bf16, kind="ExternalOutput"
    )

    with TileContext(nc) as tc:
        with (
            tc.tile_pool(name="nxt_pool", bufs=2) as nxt_pool,
            tc.tile_pool(name="prv_pool", bufs=2) as prv_pool,
            tc.tile_pool(name="band_pool", bufs=2) as band_pool,
            tc.tile_pool(name="psum_pool", bufs=2, space="PSUM") as psum_pool,
        ):
            nxt_tiles = {}

            def load_nxt(b, ring):
                n0 = nxt_pool.tile([C0, HP, WP], bf16, tag="nxt_a")
                n1 = nxt_pool.tile([C1, HP, WP], bf16, tag="nxt_b")
                for n in (n0, n1):
                    nc.gpsimd.memset(n[:, 0:R, :], 0.0)
                    nc.gpsimd.memset(n[:, R + H : HP, :], 0.0)
                for s in range(NSL):
                    lo, hi = H * s // NSL, H * (s + 1) // NSL
                    ring.dma_start(
                        n0[:, R + lo : R + hi, :], nxt_d[b, 0:C0, lo:hi, :]
                    )
                    ring.dma_start(
                        n1[:, R + lo : R + hi, :], nxt_d[b, C0:C, lo:hi, :]
                    )
                nxt_tiles[b] = (n0, n1)

            load_nxt(0, nc.scalar)

            pairs = [(b, i) for b in range(B_LOC) for i in range(NI)]
            prv_tiles = {}

            def load_prv(k):
                b, i = pairs[k]
                p0 = prv_pool.tile([C0, NJ, PH * PW], bf16, tag="prv_a")
                p1 = prv_pool.tile([C1, NJ, PH * PW], bf16, tag="prv_b")
                nc.sync.dma_start(p0[:], prv_d[b, 0:C0, i])
                nc.sync.dma_start(p1[:], prv_d[b, C0:C, i])
                prv_tiles[k] = (p0, p1)

            load_prv(0)
            load_prv(1)

            # copy-engine rotation: DVE, ACT (Pool cannot access PSUM on TRN2)
            cp_idx = 0

            for k in range(len(pairs)):
                b, i = pairs[k]
                if k + 2 < len(pairs):
                    load_prv(k + 2)
                p0, p1 = prv_tiles.pop(k)
                n0, n1 = nxt_tiles[b]
                # band64[p, J, c]: partition half 0-63 holds window cols
                # 0:256, half 64-127 holds cols 128:384 - contiguous per
                # partition, so the store is one 8KB/partition DMA.
                band = band_pool.tile([PH * PW, NJ, QW], bf16, tag="band_sb")
                r0 = slice(i * PH, i * PH + WH)
                for tq in range(NJ // 4):
                    # one 4-bank psum tile per quad (4 patches); two
                    # half-partition copies evacuate it (one per engine).
                    # Chunk-major matmul order: uniform K=64 shapes issue
                    # back-to-back on the PE at the full clock rate.
                    ps = psum_pool.tile([PH * PW, 4, 512], fp32, tag="band_ps")
                    for m in range(4):
                        j = 4 * tq + m
                        cj = slice(j * PW, j * PW + WW)
                        nc.tensor.matmul(
                            ps[:, m, 0:NB], p0[:, j, :], n0[:, r0, cj],
                            start=True, stop=False,
                        )
                    for m in range(4):
                        j = 4 * tq + m
                        cj = slice(j * PW, j * PW + WW)
                        nc.tensor.matmul(
                            ps[:, m, 0:NB], p1[:, j, :], n1[:, r0, cj],
                            start=False, stop=True,
                        )
                    ja = slice(4 * tq, 4 * tq + 4)
                    if tq % 2 == 0:
                        nc.vector.tensor_copy(
                            band[0:64, ja, :], ps[0:64, :, 0:QW]
                        )
                        nc.scalar.copy(
                            band[64:128, ja, :], ps[64:128, :, NB - QW : NB]
                        )
                    else:
                        nc.scalar.copy(band[0:64, ja, :], ps[0:64, :, 0:QW])
                        nc.vector.tensor_copy(
                            band[64:128, ja, :], ps[64:128, :, NB - QW : NB]
                        )
                ring = nc.sync if k % 2 == 0 else nc.scalar
                ring.dma_start(band_d[b, i], band[:])
                if k == 0:
                    load_nxt(1, nc.gpsimd)

    nc.finalize()
    return nc


def _get_nc():
    if "nc" not in _CACHED:
        _CACHED["nc"] = _build_nc()
    return _CACHED["nc"]


def _host_prep(prv, nxt):
    """prv: scale by 1/C, bf16, patch-major [b, c, I, J, 128].
    nxt: bf16 [b, c, 128, 136] zero-padded cols only."""
    bf16 = ml_dtypes.bfloat16
    prv_t = (np.asarray(prv, dtype=np.float32) * (1.0 / C)).transpose(0, 3, 1, 2)
    prv_t = prv_t.reshape(B, C, NI, PH, NJ, PW).transpose(0, 1, 2, 4, 3, 5)
    prv_t = np.ascontiguousarray(prv_t.reshape(B, C, NI, NJ, PH * PW)).astype(bf16)
    nxt_t = np.asarray(nxt, dtype=np.float32).transpose(0, 3, 1, 2).astype(bf16)
    nxt_p = np.zeros((B, C, H, WP), dtype=bf16)
    nxt_p[:, :, :, R : R + W] = nxt_t
    return prv_t, nxt_p


def _make_in_maps(prv, nxt):
    prv_t, nxt_p = _host_prep(prv, nxt)
    return [
        {
            "prv_t": prv_t[i * B_LOC : (i + 1) * B_LOC],
            "nxt_p": nxt_p[i * B_LOC : (i + 1) * B_LOC],
        }
        for i in range(N_CORES)
    ]


# gather index over the per-half 256-wide window:
# c[p, di, dj] = (((p>>3)&7) + di)*16 + (p&7) + dj  (uniform for both halves)
_p = np.arange(PH * PW)
_di, _dj = np.meshgrid(np.arange(D), np.arange(D), indexing="ij")
_GIDX = (
    (((_p >> 3) & 7)[:, None, None] + _di[None]) * WW
    + (_p & 7)[:, None, None]
    + _dj[None]
).reshape(1, 1, PH * PW, 1, D * D)  # [1,1,128,1,81]


def _gather_band(band8):
    """band8: [B_LOC, NI, 128, NJ, QW] bf16 -> out [B_LOC, H, W, D*D] f32."""
    arr = np.asarray(band8, dtype=np.float32)  # [b, I, p, J, QW]
    idx = np.broadcast_to(_GIDX, arr.shape[:4] + (D * D,))
    out = np.take_along_axis(arr, idx, axis=-1)  # [b, I, p, J, 81]
    out = out.reshape(B_LOC, NI, PH, PW, NJ, D * D)  # p = (i, j)
    out = out.transpose(0, 1, 2, 4, 3, 5)  # [b, I, i, J, j, 81]
    return np.ascontiguousarray(out.reshape(B_LOC, H, W, D * D))


def kernel(prv, nxt, search_range):
    from concourse.bass_utils import run_bass_kernel_spmd

    assert int(search_range) == R
    prv = np.asarray(prv)
    nxt = np.asarray(nxt)
    assert prv.shape == (B, H, W, C), prv.shape

    in_maps = _make_in_maps(prv, nxt)

    nc = _get_nc()
    res = run_bass_kernel_spmd(nc, in_maps, list(range(N_CORES)))

    out = np.empty((B, H, W, D * D), dtype=np.float32)
    for i in range(N_CORES):
        out[i * B_LOC : (i + 1) * B_LOC] = _gather_band(res.results[i]["band"])
    return out


# revision 35
# speedup vs baseline: 1.0157x; 1.0157x over previous
"""CostVolume kernel for Trainium2 (8 NeuronCores, batch-sharded).

out[b,h,w,(di,dj)] = mean_c( prv[b,h,w,c] * nxt_pad[b,h+di,w+dj,c] ),  r=4, d=9.

Device strategy (per core, 2 batches):
  - Host prep: prv scaled by 1/C -> bf16, patch-major [b, c, I, J, 128];
    nxt -> bf16, [b, c, 128, 136] (cols zero-padded by 4; rows padded on
    device via gpsimd memset of the SBUF border).
  - Per 16x8-pixel patch: matmul (M=128 pixels, N=384 = 24x16 nxt window)
    contracting c: chunk1 K=128 (start) + chunk2 K=64 (accumulate). No
    operand duplication - PE issue rate is N-columns * cycle regardless of K.
  - Band SBUF layout [128, NB=384, NJ=16] (J innermost): PSUM->SBUF copies
    write J-strided; the windowed store per 32-partition group q reads
    band[32q:32q+32, 64q:64q+192, :] = one contiguous 6144B run per
    partition (full DMA efficiency), 12.6 MB/core instead of 25.2 MB.
  - PSUM->SBUF bf16 copies rotate across DVE / ACT / Pool so the tensor
    engine never stalls on PSUM recycling (lets the PE ramp past the
    1.2 GHz mid p-state to 2.4 GHz).
  - Rings: gpsimd (SWDGE) nxt loads + memsets, sync (SP) prv loads +
    band stores, scalar/vector/gpsimd share the copies.
  - Host gathers out from band8[b,I,q,p32,nb,J] with
    nb = ((p32>>3)+di)*16 + (p32&7) + dj.
"""

import numpy as np
import ml_dtypes

B, H, W, C = 16, 128, 128, 192
R = 4
D = 2 * R + 1  # 9
N_CORES = 8
B_LOC = B // N_CORES  # 2
# PE moving-read is quantized to ceil(K/64) beats per column, so split
# C=192 as K=128 (2 beats) + K=64 (1 beat) and group same-K matmuls.
C0 = 128
C1 = C - C0  # 64
PH, PW = 16, 8  # patch size (h, w); PH*PW = 128 = M
WH, WW = PH + 2 * R, PW + 2 * R  # 24, 16 window
NB = WH * WW  # 384 band columns per patch
NI = H // PH  # 8 patch rows
NJ = W // PW  # 16 patch cols
HP = H + 2 * R  # 136 padded rows (SBUF)
WP = W + 2 * R  # 136 padded cols (HBM + SBUF)
QW = 256  # band window width per 64-partition half (contiguous in SBUF)
NSL = 4  # h-slices per nxt load

_CACHED = {}


def _build_nc():
    import concourse.mybir as mybir
    from concourse.bacc import Bacc
    from concourse.tile import TileContext

    fp32 = mybir.dt.float32
    bf16 = mybir.dt.bfloat16

    nc = Bacc(
        "TRN2",
        target_bir_lowering=False,
        debug=False,
        num_devices=N_CORES,
    )

    prv_d = nc.dram_tensor(
        "prv_t", [B_LOC, C, NI, NJ, PH * PW], bf16, kind="ExternalInput"
    )
    nxt_d = nc.dram_tensor("nxt_p", [B_LOC, C, H, WP], bf16, kind="ExternalInput")
    band_d = nc.dram_tensor(
        "band", [B_LOC, NI, PH * PW, NJ, QW], bf16, kind="ExternalOutput"
    )

    with TileContext(nc) as tc:
        with (
            tc.tile_pool(name="nxt_pool", bufs=2) as nxt_pool,
            tc.tile_pool(name="prv_pool", bufs=2) as prv_pool,
            tc.tile_pool(name="band_pool", bufs=2) as band_pool,
            tc.tile_pool(name="psum_pool", bufs=4, space="PSUM") as psum_pool,
        ):
            nxt_tiles = {}

            def load_nxt(b, ring):
                n0 = nxt_pool.tile([C0, HP, WP], bf16, tag="nxt_a")
                n1 = nxt_pool.tile([C1, HP, WP], bf16, tag="nxt_b")
                for n in (n0, n1):
                    nc.gpsimd.memset(n[:, 0:R, :], 0.0)
                    nc.gpsimd.memset(n[:, R + H : HP, :], 0.0)
                for s in range(NSL):
                    lo, hi = H * s // NSL, H * (s + 1) // NSL
                    ring.dma_start(
                        n0[:, R + lo : R + hi, :], nxt_d[b, 0:C0, lo:hi, :]
                    )
                    ring.dma_start(
                        n1[:, R + lo : R + hi, :], nxt_d[b, C0:C, lo:hi, :]
                    )
                nxt_tiles[b] = (n0, n1)

            load_nxt(0, nc.scalar)

            pairs = [(b, i) for b in range(B_LOC) for i in range(NI)]
            prv_tiles = {}

            def load_prv(k):
                b, i = pairs[k]
                p0 = prv_pool.tile([C0, NJ, PH * PW], bf16, tag="prv_a")
                p1 = prv_pool.tile([C1, NJ, PH * PW], bf16, tag="prv_b")
                nc.sync.dma_start(p0[:], prv_d[b, 0:C0, i])
                nc.scalar.dma_start(p1[:], prv_d[b, C0:C, i])
                prv_tiles[k] = (p0, p1)

            load_prv(0)
            load_prv(1)

            # copy-engine rotation: DVE, ACT (Pool cannot access PSUM on TRN2)
            cp_idx = 0

            for k in range(len(pairs)):
                b, i = pairs[k]
                if k + 2 < len(pairs):
                    load_prv(k + 2)
                p0, p1 = prv_tiles.pop(k)
                n0, n1 = nxt_tiles[b]
                # band64[p, J, c]: partition half 0-63 holds window cols
                # 0:256, half 64-127 holds cols 128:384 - contiguous per
                # partition, so the store is one 8KB/partition DMA.
                band = band_pool.tile([PH * PW, NJ, QW], bf16, tag="band_sb")
                r0 = slice(i * PH, i * PH + WH)
                # groups of 4 pairs (8 patches): 8 K=128 matmuls, then 8
                # K=64 matmuls (long same-shape runs pipeline on the PE),
                # then two half-partition copies per pair.
                for g in range(2):
                    pst = []
                    for _t in range(4):
                        ps_t = psum_pool.tile(
                            [PH * PW, 2, 512], fp32, tag="band_ps"
                        )
                        pst.append(ps_t)
                    for t in range(4):
                        for m in range(2):
                            j = 8 * g + 2 * t + m
                            cj = slice(j * PW, j * PW + WW)
                            nc.tensor.matmul(
                                pst[t][:, m, 0:NB], p0[:, j, :], n0[:, r0, cj],
                                start=True, stop=False,
                            )
                    for t in range(4):
                        for m in range(2):
                            j = 8 * g + 2 * t + m
                            cj = slice(j * PW, j * PW + WW)
                            nc.tensor.matmul(
                                pst[t][:, m, 0:NB], p1[:, j, :], n1[:, r0, cj],
                                start=False, stop=True,
                            )
                    for t in range(4):
                        ja = slice(8 * g + 2 * t, 8 * g + 2 * t + 2)
                        ea, eb = (
                            (nc.vector, nc.scalar)
                            if t % 2 == 0
                            else (nc.scalar, nc.vector)
                        )
                        (ea.copy if ea is nc.scalar else ea.tensor_copy)(
                            band[0:64, ja, :], pst[t][0:64, :, 0:QW]
                        )
                        (eb.copy if eb is nc.scalar else eb.tensor_copy)(
                            band[64:128, ja, :], pst[t][64:128, :, NB - QW : NB]
                        )
                nc.sync.dma_start(band_d[b, i], band[:])
                if k == 0:
                    load_nxt(1, nc.gpsimd)

    nc.finalize()
    return nc


def _get_nc():
    if "nc" not in _CACHED:
        _CACHED["nc"] = _build_nc()
    return _CACHED["nc"]


def _host_prep(prv, nxt):
    """prv: scale by 1/C, bf16, patch-major [b, c, I, J, 128].
    nxt: bf16 [b, c, 128, 136] zero-padded cols only."""
    bf16 = ml_dtypes.bfloat16
    prv_t = (np.asarray(prv, dtype=np.float32) * (1.0 / C)).transpose(0, 3, 1, 2)
    prv_t = prv_t.reshape(B, C, NI, PH, NJ, PW).transpose(0, 1, 2, 4, 3, 5)
    prv_t = np.ascontiguousarray(prv_t.reshape(B, C, NI, NJ, PH * PW)).astype(bf16)
    nxt_t = np.asarray(nxt, dtype=np.float32).transpose(0, 3, 1, 2).astype(bf16)
    nxt_p = np.zeros((B, C, H, WP), dtype=bf16)
    nxt_p[:, :, :, R : R + W] = nxt_t
    return prv_t, nxt_p


def _make_in_maps(prv, nxt):
    prv_t, nxt_p = _host_prep(prv, nxt)
    return [
        {
            "prv_t": prv_t[i * B_LOC : (i + 1) * B_LOC],
            "nxt_p": nxt_p[i * B_LOC : (i + 1) * B_LOC],
        }
        for i in range(N_CORES)
    ]


# gather index over the per-half 256-wide window:
# c[p, di, dj] = (((p>>3)&7) + di)*16 + (p&7) + dj  (uniform for both halves)
_p = np.arange(PH * PW)
_di, _dj = np.meshgrid(np.arange(D), np.arange(D), indexing="ij")
_GIDX = (
    (((_p >> 3) & 7)[:, None, None] + _di[None]) * WW
    + (_p & 7)[:, None, None]
    + _dj[None]
).reshape(1, 1, PH * PW, 1, D * D)  # [1,1,128,1,81]


def _gather_band(band8):
    """band8: [B_LOC, NI, 128, NJ, QW] bf16 -> out [B_LOC, H, W, D*D] f32."""
    arr = np.asarray(band8, dtype=np.float32)  # [b, I, p, J, QW]
    idx = np.broadcast_to(_GIDX, arr.shape[:4] + (D * D,))
    out = np.take_along_axis(arr, idx, axis=-1)  # [b, I, p, J, 81]
    out = out.reshape(B_LOC, NI, PH, PW, NJ, D * D)  # p = (i, j)
    out = out.transpose(0, 1, 2, 4, 3, 5)  # [b, I, i, J, j, 81]
    return np.ascontiguousarray(out.reshape(B_LOC, H, W, D * D))


def kernel(prv, nxt, search_range):
    from concourse.bass_utils import run_bass_kernel_spmd

    assert int(search_range) == R
    prv = np.asarray(prv)
    nxt = np.asarray(nxt)
    assert prv.shape == (B, H, W, C), prv.shape

    in_maps = _make_in_maps(prv, nxt)

    nc = _get_nc()
    res = run_bass_kernel_spmd(nc, in_maps, list(range(N_CORES)))

    out = np.empty((B, H, W, D * D), dtype=np.float32)
    for i in range(N_CORES):
        out[i * B_LOC : (i + 1) * B_LOC] = _gather_band(res.results[i]["band"])
    return out
